# revision 1
# baseline (speedup 1.0000x reference)
"""GCNEncoder Trainium2 kernel (8 NeuronCores, SPMD).

Strategy (graph/data parallel, per sharding hint):
  - Nodes are dealt round-robin-by-degree across 8 cores (2500 each); the
    [H,H] weights are replicated.
  - Per GCN layer: each core scales its node rows by dinv=1/sqrt(deg), casts
    to bf16 and AllGathers the full 20000x256 "table" into every core's HBM.
  - Message aggregation = segment-sum over in-edges:  per 128-destination
    group, a transposed dma_gather pulls the source rows (feature-major:
    [128h, 2, 128*K]) and a strided DVE reduce sums each destination's K
    slots (padding slots point at an all-zero table row).
  - The GCNConv reorder agg(x) @ W == agg(x @ W) lets one aggregation per
    layer feed the [HxH] matmul afterwards; out2/out3 share the layer-3
    aggregation.  norm = dinv[row]*dinv[col] factorizes into the table
    pre-scale and a per-destination post-scale fused into the PSUM->SBUF
    activation (bias is added via a K=1 rank-1 matmul of sqrt(deg) x b).

Self-contained: hardcodes the problem shapes; only needs numpy + concourse.
"""

import numpy as np

# -------------------- problem constants --------------------
N_NODES = 20000
N_EDGES = 320000
H = 256
C = 8  # cores

_KERNEL_CACHE = {}
LAST_RESULTS = None  # BassKernelResults of the most recent run (for profiling)


# -------------------- host-side graph prep --------------------
def _prep_graph(edge_index, n_nodes, n_cores):
    """Partition nodes, build per-core padded gather-slot index arrays.

    Returns dict with permutation, per-core degree arrays, gather indices.
    """
    P = n_nodes // n_cores  # nodes per core
    row = edge_index[0].astype(np.int64)
    col = edge_index[1].astype(np.int64)
    loop = np.arange(n_nodes, dtype=np.int64)
    row_f = np.concatenate([row, loop])
    col_f = np.concatenate([col, loop])
    deg = np.bincount(col_f, minlength=n_nodes).astype(np.int64)  # >= 1

    # deal nodes round-robin by ascending degree -> every core gets an
    # almost identical degree profile, sorted ascending within the core.
    order = np.argsort(deg, kind="stable")
    pos = np.empty(n_nodes, dtype=np.int64)
    pos[order] = np.arange(n_nodes)
    new_id = (pos % n_cores) * P + pos // n_cores  # old -> new
    orig_of_new = np.empty(n_nodes, dtype=np.int64)
    orig_of_new[new_id] = np.arange(n_nodes)

    src_new = new_id[row_f]
    dst_new = new_id[col_f]

    PT = ((P + 127) // 128) * 128  # padded dest count per core
    NG = PT // 128  # 128-dest groups
    PR = P + 16  # table rows contributed per rank (16 zero pad rows)
    ZROW = P  # rank0's first pad row: an all-zero table row

    deg_new = deg[orig_of_new]  # per new id
    # per-core local degree arrays, padded with 1.0
    deg_loc = np.ones((n_cores, PT), dtype=np.float32)
    for c in range(n_cores):
        deg_loc[c, :P] = deg_new[c * P : (c + 1) * P]

    # max (over cores) degree within a local-dest range, %4-rounded
    # (clean 2x-mode DVE pair-adds need K%4: even halves at 4B alignment)
    def range_K(lo, hi):
        m = 0
        for c in range(n_cores):
            a, b = c * P + lo, c * P + min(hi, P)
            if a < b:
                m = max(m, int(deg_new[a:b].max()))
        return max(((m + 3) // 4) * 4, 4)

    # gather chunks: whole-dest sub-ranges of each 128-dest group, <= MAXI
    # indices per dma_gather (descriptor-ring limit at 48KB DMA scratch);
    # each chunk gets its own K to minimise zero-slot padding
    MAXI = 2944
    Kg = []
    chunks = []  # (dest_off_in_core, dc, K, idx_off)
    ioff = 0
    for g in range(NG):
        gK = range_K(g * 128, (g + 1) * 128)
        Kg.append(gK)
        dc = 128
        while dc * gK > MAXI:
            dc //= 2
        assert dc >= 32
        for d0 in range(0, 128, dc):
            cK = range_K(g * 128 + d0, g * 128 + d0 + dc)
            assert (dc * cK) % 128 == 0
            chunks.append((g * 128 + d0, dc, cK, ioff))
            ioff += dc * cK
    TOT = int(ioff)  # slots per core (same for all cores)

    # per-dest slot base/K for filling
    dest_base = np.zeros(PT, dtype=np.int64)
    dest_K = np.ones(PT, dtype=np.int64)
    for doff, dc, cK, io in chunks:
        d = np.arange(dc)
        dest_base[doff : doff + dc] = io + d * cK
        dest_K[doff : doff + dc] = cK

    # slot array [cores, TOT] filled with ZROW, then scatter edge sources.
    # table row of new node id n = (n // P) * PR + (n % P)
    src_trow = (src_new // P) * PR + (src_new % P)
    slots = np.full((n_cores, TOT), ZROW, dtype=np.int64)
    e_core = dst_new // P
    e_dloc = dst_new % P
    sort_k = np.argsort(e_core * n_nodes + e_dloc, kind="stable")
    sc, sd, ss = e_core[sort_k], e_dloc[sort_k], src_trow[sort_k]
    # rank within each (core,dest) run
    key = sc * n_nodes + sd
    first = np.r_[True, key[1:] != key[:-1]]
    run_start = np.maximum.accumulate(np.where(first, np.arange(key.size), 0))
    rank = np.arange(key.size) - run_start
    flat = dest_base[sd] + rank
    slots[sc, flat] = ss

    # wrap to [128, TOT//16] int16: element (p, s) = slots[s*16 + p%16]
    assert TOT % 16 == 0
    wrapped = np.empty((n_cores, 128, TOT // 16), dtype=np.int16)
    for c in range(n_cores):
        w16 = slots[c].reshape(TOT // 16, 16).T.astype(np.int16)  # [16, TOT/16]
        wrapped[c] = np.tile(w16, (8, 1))

    return dict(
        P=P, PT=PT, NG=NG, TOT=TOT, ZROW=ZROW, PR=PR,
        Kg=[int(k) for k in Kg], offs=[0],
        chunks=chunks,
        new_id=new_id, orig_of_new=orig_of_new,
        deg_loc=deg_loc, gidx=wrapped,
    )


# -------------------- bass kernel builder --------------------
def _build_bass(n_nodes, n_cores, h, P, PT, NG, TOT, Kg, offs, PR, chunks,
                repeat=1, collective=True):
    import concourse.bass as bass
    import concourse.bacc as bacc
    import concourse.mybir as mybir
    import concourse.tile as tile
    from concourse import library_config

    dt = mybir.dt
    f32, bf16, i16 = dt.float32, dt.bfloat16, dt.int16
    AF = mybir.ActivationFunctionType
    NT = PT // 128  # node tiles per core
    NTAB = n_cores * PR  # table rows (rank r at [r*PR, r*PR+P); pads zero)
    KC = h // 128  # contraction chunks (2)

    nc = bacc.Bacc(dynamic_dma_scratch_size=49152)
    x_in = nc.declare_dram_parameter("x_shard", [P, h], f32, isOutput=False)
    deg_in = nc.declare_dram_parameter("deg_loc", [PT], f32, isOutput=False)
    idx_in = nc.declare_dram_parameter("gidx", [128, TOT // 16], i16, isOutput=False)
    W_in = [nc.declare_dram_parameter(nm, [h, h], f32, isOutput=False)
            for nm in ("W1", "W1_1", "W2", "W3")]
    b_in = [nc.declare_dram_parameter(nm, [h], f32, isOutput=False)
            for nm in ("b1", "b1_1", "b2", "b3")]
    out2_ext = nc.declare_dram_parameter("out2", [P, h], f32, isOutput=True)
    out3_ext = nc.declare_dram_parameter("out3", [P, h], f32, isOutput=True)

    with tile.TileContext(nc) as tc:
        with (
            tc.tile_pool(name="dram", bufs=1, space="DRAM") as dpool,
            tc.tile_pool(name="const", bufs=1) as cpool,
            tc.tile_pool(name="gather", bufs=4) as gpool,
            tc.tile_pool(name="rbuf", bufs=6) as rpool,
            tc.tile_pool(name="work", bufs=4) as wpool,
            tc.tile_pool(name="psum", bufs=8, space="PSUM") as ppool,
        ):
            # ---- internal DRAM ---- (per-repeat for benchmark variants:
            # Tile requires a single writer for Shared DRAM)
            ag_in_r = [
                [dpool.tile([PR, h], bf16, name=f"agin{L}_{r}") for L in range(3)]
                for r in range(repeat)
            ]
            if collective:
                tables_r = [
                    [dpool.tile([NTAB, h], bf16, addr_space="Shared",
                                name=f"table{L}_{r}") for L in range(3)]
                    for r in range(repeat)
                ]
            else:  # timing-study variant: tables fed as plain inputs, no AG
                tin = [
                    nc.declare_dram_parameter(f"tbl{L}", [NTAB, h], bf16,
                                              isOutput=False)
                    for L in range(3)
                ]
                tables_r = [tin for _ in range(repeat)]

            # ---- constants ----
            w_sb = []
            for i in range(4):
                wt = cpool.tile([128, KC, h], f32, name=f"w{i}")
                nc.sync.dma_start(wt[:], W_in[i].rearrange("(c p) j -> p c j", p=128))
                wb = cpool.tile([128, KC, h], bf16, name=f"wb{i}")
                nc.vector.tensor_copy(wb[:], wt[:])
                w_sb.append(wb)
            b_sb = []
            for i in range(4):
                bt = cpool.tile([1, h], f32, name=f"bv{i}")
                nc.sync.dma_start(bt[:], b_in[i][None, :])
                b_sb.append(bt)

            deg_row = cpool.tile([1, PT], f32, name="deg_row")
            nc.sync.dma_start(deg_row[:], deg_in[None, :])
            sqd_row = cpool.tile([1, PT], f32, name="sqd_row")
            nc.scalar.sqrt(sqd_row[:], deg_row[:])

            deg_nm = cpool.tile([128, NT], f32, name="deg_nm")
            nc.sync.dma_start(deg_nm[:], deg_in.rearrange("(t p) -> p t", p=128))
            sq_nm = cpool.tile([128, NT], f32, name="sq_nm")
            nc.scalar.sqrt(sq_nm[:], deg_nm[:])
            dinv_nm = cpool.tile([128, NT], f32, name="dinv_nm")
            nc.vector.reciprocal(dinv_nm[:], sq_nm[:])
            dinv2_nm = cpool.tile([128, NT], f32, name="dinv2_nm")
            nc.vector.tensor_mul(dinv2_nm[:], dinv_nm[:], dinv_nm[:])

            gidx = cpool.tile([128, TOT // 16], i16, name="gidx_sb")
            nc.sync.dma_start(gidx[:], idx_in[:])

            rg = [list(range(n_cores))]
            zpad = cpool.tile([PR - P, h], bf16, name="zpad")
            nc.vector.memset(zpad[:], 0.0)

            # chunks grouped by 128-dest tile
            by_group = [[] for _ in range(NG)]
            for ch in chunks:
                by_group[ch[0] // 128].append(ch)

            def mm_into(ps, Rb, t, wi, start=True):
                for c in range(KC):
                    nc.tensor.matmul(
                        ps[:],
                        lhsT=Rb[:, c, :],
                        rhs=w_sb[wi][:, c, :],
                        start=(start and c == 0),
                        stop=False,
                    )
                nc.tensor.matmul(
                    ps[:],
                    lhsT=sqd_row[0:1, t * 128 : (t + 1) * 128],
                    rhs=b_sb[wi][:],
                    start=False,
                    stop=True,
                )

            def process_layer(rep, L):
                """AllGather table L, then per 128-dest group: gather in-edge
                rows, tree-reduce on DVE, matmul + fused epilogue, emit either
                the next layer's AG input (L<2) or the two output heads."""
                ag_in = ag_in_r[rep]
                if collective:
                    nc.gpsimd.collective_compute(
                        "AllGather",
                        mybir.AluOpType.bypass,
                        replica_groups=rg,
                        ins=[ag_in[L].opt()],
                        outs=[tables_r[rep][L].opt()],
                    )
                # biggest groups first: the layer tail (which gates the next
                # AllGather) then drains through the cheapest chunks
                for g in sorted(range(NG), key=lambda gg: -Kg[gg]):
                    Rg = rpool.tile([128, KC, 128], f32, tag="Rg",
                                    name=f"Rg{rep}_{L}_{g}")
                    for ci, (doff, dc, K, ioff) in enumerate(by_group[g]):
                        n_idx = dc * K
                        gt = gpool.tile([128, KC, n_idx], bf16, tag="gt",
                                        name=f"gt{rep}_{L}_{g}_{ci}")
                        nc.gpsimd.dma_gather(
                            gt[:],
                            tables_r[rep][L][:, :],
                            gidx[:, ioff // 16 : (ioff + n_idx) // 16],
                            n_idx,
                            n_idx,
                            h,
                            transpose=True,
                            single_packet=(n_idx <= 896),
                        )
                        # in-place pair-add halving while 2x-mode legal
                        # (runs even + 4B-aligned bases requires K' % 4 == 0)
                        cK = K
                        g4 = gt.rearrange("p c (d k) -> p c d k", k=K)
                        while cK % 4 == 0 and cK > 2:
                            nh = cK // 2
                            nc.vector.tensor_add(
                                g4[:, :, :, 0:nh],
                                g4[:, :, :, 0:nh],
                                g4[:, :, :, nh:cK],
                            )
                            cK = nh
                        nc.vector.tensor_reduce(
                            Rg[:, :, doff % 128 : doff % 128 + dc],
                            g4[:, :, :, 0:cK],
                            axis=mybir.AxisListType.X,
                            op=mybir.AluOpType.add,
                        )
                    Rb = rpool.tile([128, KC, 128], bf16, tag="Rbg",
                                    name=f"Rb{rep}_{L}_{g}")
                    nc.scalar.copy(Rb[:], Rg[:])  # ACT: f32 -> bf16 for the PE
                    rows = min(128, P - g * 128)
                    if L < 2:
                        ps = ppool.tile([128, h], f32, tag="ps",
                                        name=f"ps{rep}_{L}_{g}")
                        mm_into(ps, Rb, g, L)
                        # T = dinv*relu(dinv*(RW) + b) = relu(dinv^2*psum)
                        tt = wpool.tile([128, h], bf16, tag="tt",
                                        name=f"ttl{rep}_{L}_{g}")
                        nc.scalar.activation(
                            tt[:], ps[:], AF.Relu, scale=dinv2_nm[:, g : g + 1]
                        )
                        nc.sync.dma_start(
                            ag_in[L + 1][g * 128 : g * 128 + rows, :], tt[:rows, :]
                        )
                    else:
                        ps2 = ppool.tile([128, h], f32, tag="ps",
                                         name=f"ps2_{rep}_{g}")
                        mm_into(ps2, Rb, g, 2)
                        ps3 = ppool.tile([128, h], f32, tag="ps",
                                         name=f"ps3_{rep}_{g}")
                        mm_into(ps3, Rb, g, 3)
                        o2 = wpool.tile([128, h], f32, tag="hsb",
                                        name=f"o2_{rep}_{g}")
                        nc.scalar.activation(
                            o2[:], ps2[:], AF.Copy, scale=dinv_nm[:, g : g + 1]
                        )
                        nc.sync.dma_start(
                            out2_ext[g * 128 : g * 128 + rows, :], o2[:rows, :]
                        )
                        o3 = wpool.tile([128, h], f32, tag="hsb",
                                        name=f"o3_{rep}_{g}")
                        nc.scalar.activation(
                            o3[:], ps3[:], AF.Copy, scale=dinv_nm[:, g : g + 1]
                        )
                        nc.sync.dma_start(
                            out3_ext[g * 128 : g * 128 + rows, :], o3[:rows, :]
                        )

            for rep in range(repeat):
                ag_in = ag_in_r[rep]
                for L in range(3):
                    nc.sync.dma_start(ag_in[L][P:PR, :], zpad[:])

                # ---- prologue: T1 = bf16(dinv * x) on ACT ----
                for t in range(NT):
                    rows = min(128, P - t * 128)
                    xt = wpool.tile([128, h], f32, tag="xt", name=f"xt{rep}_{t}")
                    nc.sync.dma_start(xt[:rows, :], x_in[t * 128 : t * 128 + rows, :])
                    tt = wpool.tile([128, h], bf16, tag="tt", name=f"tt{rep}_{t}")
                    nc.scalar.activation(
                        tt[:rows, :], xt[:rows, :], AF.Copy,
                        scale=dinv_nm[:rows, t : t + 1],
                    )
                    nc.sync.dma_start(
                        ag_in[0][t * 128 : t * 128 + rows, :], tt[:rows, :]
                    )

                for L in range(3):
                    process_layer(rep, L)

    nc.compile()
    return nc


# -------------------- public entry --------------------
def kernel(x, edge_index, W1, b1, W1_1, b1_1, W2, b2, W3, b3):
    from concourse.bass_utils import run_bass_kernel_spmd

    x = np.asarray(x, dtype=np.float32)
    edge_index = np.asarray(edge_index, dtype=np.int32)
    n_nodes, h = x.shape
    meta = _prep_graph(edge_index, n_nodes, C)
    P, PT, NG, TOT = meta["P"], meta["PT"], meta["NG"], meta["TOT"]

    key = (n_nodes, h, tuple(meta["Kg"]))
    if key not in _KERNEL_CACHE:
        _KERNEL_CACHE[key] = _build_bass(
            n_nodes, C, h, P, PT, NG, TOT, meta["Kg"], meta["offs"], meta["PR"],
            meta["chunks"],
        )
    nc = _KERNEL_CACHE[key]

    oon = meta["orig_of_new"]
    Ws = {"W1": W1, "W1_1": W1_1, "W2": W2, "W3": W3}
    bs = {"b1": b1, "b1_1": b1_1, "b2": b2, "b3": b3}
    in_maps = []
    for c in range(C):
        m = {
            "x_shard": np.ascontiguousarray(
                x[oon[c * P : (c + 1) * P]], dtype=np.float32
            ),
            "deg_loc": meta["deg_loc"][c],
            "gidx": np.ascontiguousarray(meta["gidx"][c]),
        }
        for k, v in Ws.items():
            m[k] = np.ascontiguousarray(v, dtype=np.float32)
        for k, v in bs.items():
            m[k] = np.ascontiguousarray(v, dtype=np.float32)
        in_maps.append(m)

    global LAST_RESULTS
    LAST_RESULTS = run_bass_kernel_spmd(nc, in_maps, core_ids=list(range(C)))
    res = LAST_RESULTS.results

    out2_new = np.concatenate([res[c]["out2"] for c in range(C)], axis=0)
    out3_new = np.concatenate([res[c]["out3"] for c in range(C)], axis=0)
    new_id = meta["new_id"]
    return out2_new[new_id].astype(np.float32), out3_new[new_id].astype(np.float32)



# revision 2
# speedup vs baseline: 1.2467x; 1.2467x over previous
"""GCNEncoder Trainium2 kernel (8 NeuronCores, SPMD) — v2.

Strategy (graph/data parallel, per sharding hint):
  - Nodes dealt round-robin-by-degree across 8 cores (2500 each); [H,H]
    weights replicated (shipped bf16).
  - Layer-1 table (dinv * x, bf16) is computed host-side and passed as an
    input, so only 2 AllGathers remain (out2/out3 share the layer-3 agg).
  - Per layer: gather in-edge source rows from the bf16 table with
    transposed dma_gather (feature-major [128, 2, slots]); slots laid out
    in 16-destination windows with exact per-window K = max in-degree, so
    padding is minimal; each gather call (<=3968 idxs) spans many windows.
  - Segment sum on DVE: ceil-halving fold chains (2x-mode adds; top half
    folded onto bottom half, odd middle slot passes through), final pair
    written into the layer R tile (f32, feature-major).  One DVE op per
    fold level per same-K run, so op count stays small.
  - agg(x) @ W == agg(x @ W): one aggregation per layer feeds the [HxH]
    matmul; norm = dinv[row]*dinv[col] folds into the table pre-scale and
    a per-dest post-scale fused into the PSUM->SBUF activation; bias
    enters as a rank-1 (sqrt(deg) x b) matmul.

Self-contained: hardcodes problem shapes; needs only numpy + concourse.
"""

import numpy as np

# -------------------- problem constants --------------------
N_NODES = 20000
N_EDGES = 320000
H = 256
C = 8  # cores
WIN = 16  # dests per K-window
MAXI = 3968  # max idxs per dma_gather call (descriptor ring, 64KB scratch)
SCRATCH = 65536

_KERNEL_CACHE = {}
LAST_RESULTS = None


# -------------------- host-side graph prep --------------------
def _prep_graph(edge_index, n_nodes, n_cores):
    P = n_nodes // n_cores
    row = edge_index[0].astype(np.int64)
    col = edge_index[1].astype(np.int64)
    loop = np.arange(n_nodes, dtype=np.int64)
    row_f = np.concatenate([row, loop])
    col_f = np.concatenate([col, loop])
    deg = np.bincount(col_f, minlength=n_nodes).astype(np.int64)  # >= 1

    # deal nodes round-robin by ascending degree
    order = np.argsort(deg, kind="stable")
    pos = np.empty(n_nodes, dtype=np.int64)
    pos[order] = np.arange(n_nodes)
    new_id = (pos % n_cores) * P + pos // n_cores  # old -> new
    orig_of_new = np.empty(n_nodes, dtype=np.int64)
    orig_of_new[new_id] = np.arange(n_nodes)

    src_new = new_id[row_f]
    dst_new = new_id[col_f]

    PT = ((P + 127) // 128) * 128
    NW = PT // WIN
    PR = P + 16  # table rows per rank (16 zero pad rows)
    ZROW = P  # rank0's first pad row == all-zero table row
    NTAB = n_cores * PR

    deg_new = deg[orig_of_new]
    deg_loc = np.ones((n_cores, PT), dtype=np.float32)
    for c in range(n_cores):
        deg_loc[c, :P] = deg_new[c * P : (c + 1) * P]

    # per-window K: max degree over all cores in the window (exact)
    dl = np.ones((n_cores, PT), dtype=np.int64)
    for c in range(n_cores):
        dl[c, :P] = deg_new[c * P : (c + 1) * P]
    Kw = np.zeros(NW, dtype=np.int64)
    for w in range(NW):
        Kw[w] = max(int(dl[:, w * WIN : (w + 1) * WIN].max()), 2)

    # stream: natural window order (ascending K).  The last gather call then
    # holds the few highest-K windows (~1 dest group), so the per-layer
    # drain (final fold -> matmul -> write chain) is short.
    stream = list(range(NW))

    # pack gather calls: whole windows, n_idx <= MAXI, pad to %128 with ZROW
    calls_raw = []
    cur, cur_n = [], 0
    for w in stream:
        wn = WIN * int(Kw[w])
        if cur_n + wn > MAXI and cur:
            calls_raw.append((cur_n + (-cur_n) % 128, cur))
            cur, cur_n = [], 0
        cur.append((w, cur_n))
        cur_n += wn
    if cur:
        calls_raw.append((cur_n + (-cur_n) % 128, cur))

    # slot stream offsets per window
    TOT = 0
    woff = {}
    for n_idx, ws in calls_raw:
        for w, off in ws:
            woff[w] = TOT + off
        TOT += n_idx
    dest_base = np.zeros(PT, dtype=np.int64)
    for w in range(NW):
        d = np.arange(WIN)
        dest_base[w * WIN : (w + 1) * WIN] = woff[w] + d * Kw[w]

    # segments: same-K ascending-window runs within one call
    calls = []
    for n_idx, ws in calls_raw:
        segs = []
        i = 0
        while i < len(ws):
            w0, off0 = ws[i]
            j = i
            while (
                j + 1 < len(ws)
                and ws[j + 1][0] == ws[j][0] + 1
                and Kw[ws[j + 1][0]] == Kw[w0]
            ):
                j += 1
            segs.append((off0, j - i + 1, int(Kw[w0]), w0 * WIN))
            i = j + 1
        calls.append((int(n_idx), segs))

    # fill slots
    slots = np.full((n_cores, TOT), ZROW, dtype=np.int64)
    src_trow = (src_new // P) * PR + (src_new % P)
    e_core = dst_new // P
    e_dloc = dst_new % P
    sort_k = np.argsort(e_core * n_nodes + e_dloc, kind="stable")
    sc, sd, ss = e_core[sort_k], e_dloc[sort_k], src_trow[sort_k]
    key = sc * n_nodes + sd
    first = np.r_[True, key[1:] != key[:-1]]
    run_start = np.maximum.accumulate(np.where(first, np.arange(key.size), 0))
    rank = np.arange(key.size) - run_start
    flat = dest_base[sd] + rank
    slots[sc, flat] = ss

    # wrap to [128, TOT//16] int16
    assert TOT % 16 == 0
    wrapped = np.empty((n_cores, 128, TOT // 16), dtype=np.int16)
    for c in range(n_cores):
        w16 = slots[c].reshape(TOT // 16, 16).T.astype(np.int16)
        wrapped[c] = np.tile(w16, (8, 1))

    return dict(
        P=P, PT=PT, NW=NW, TOT=TOT, ZROW=ZROW, PR=PR, NTAB=NTAB,
        Kw=[int(k) for k in Kw], calls=calls,
        new_id=new_id, orig_of_new=orig_of_new,
        deg_loc=deg_loc, gidx=wrapped,
    )


# -------------------- bass kernel builder --------------------
def _build_bass(n_nodes, n_cores, h, P, PT, TOT, PR, NTAB, calls,
                collective=True):
    import concourse.bacc as bacc
    import concourse.mybir as mybir
    import concourse.tile as tile

    dt = mybir.dt
    f32, bf16, i16 = dt.float32, dt.bfloat16, dt.int16
    AF = mybir.ActivationFunctionType
    NT = PT // 128  # 128-dest groups per core
    KC = h // 128  # 2

    nc = bacc.Bacc(dynamic_dma_scratch_size=SCRATCH)
    tbl0_in = nc.declare_dram_parameter("table0", [NTAB, h], bf16, isOutput=False)
    deg_in = nc.declare_dram_parameter("deg_loc", [PT], f32, isOutput=False)
    idx_in = nc.declare_dram_parameter("gidx", [128, TOT // 16], i16, isOutput=False)
    W_in = [nc.declare_dram_parameter(nm, [h, h], bf16, isOutput=False)
            for nm in ("W1", "W1_1", "W2", "W3")]
    b_in = [nc.declare_dram_parameter(nm, [h], bf16, isOutput=False)
            for nm in ("b1", "b1_1", "b2", "b3")]
    out2_ext = nc.declare_dram_parameter("out2", [P, h], bf16, isOutput=True)
    out3_ext = nc.declare_dram_parameter("out3", [P, h], bf16, isOutput=True)

    # per-group segment counts (for emit scheduling)
    seg_cnt = [0] * NT
    for n_idx, segs in calls:
        for soff, nw, K, dest_start in segs:
            g0 = dest_start // 128
            g1 = (dest_start + nw * WIN - 1) // 128
            for g in range(g0, g1 + 1):
                seg_cnt[g] += 1

    with tile.TileContext(nc) as tc:
        with (
            tc.tile_pool(name="dram", bufs=1, space="DRAM") as dpool,
            tc.tile_pool(name="const", bufs=1) as cpool,
            tc.tile_pool(name="gather", bufs=4) as gpool,
            tc.tile_pool(name="rlay", bufs=2) as rpool,
            tc.tile_pool(name="rbg", bufs=4) as rbpool,
            tc.tile_pool(name="work", bufs=4) as wpool,
            tc.tile_pool(name="psum", bufs=8, space="PSUM") as ppool,
        ):
            # gidx first: the first gather depends only on its own slice
            gidx = cpool.tile([128, TOT // 16], i16, name="gidx_sb")
            n0 = calls[0][0]
            nc.sync.dma_start(gidx[:, : n0 // 16], idx_in[:, : n0 // 16])
            nc.sync.dma_start(gidx[:, n0 // 16 :], idx_in[:, n0 // 16 :])

            # internal DRAM: AG inputs (layer 1,2 tables)
            ag_in = [dpool.tile([PR, h], bf16, name=f"agin{L}") for L in (1, 2)]
            if collective:
                tables = [None,
                          dpool.tile([NTAB, h], bf16, addr_space="Shared",
                                     name="table1"),
                          dpool.tile([NTAB, h], bf16, addr_space="Shared",
                                     name="table2")]
            else:
                tables = [None,
                          nc.declare_dram_parameter("tbl1", [NTAB, h], bf16,
                                                    isOutput=False),
                          nc.declare_dram_parameter("tbl2", [NTAB, h], bf16,
                                                    isOutput=False)]

            # ---- constants ----
            w_sb = []
            for i in range(4):
                wb = cpool.tile([128, KC, h], bf16, name=f"wb{i}")
                nc.sync.dma_start(wb[:], W_in[i].rearrange("(c p) j -> p c j", p=128))
                w_sb.append(wb)
            b_sb = []
            for i in range(4):
                bt = cpool.tile([1, h], bf16, name=f"bv{i}")
                nc.sync.dma_start(bt[:], b_in[i][None, :])
                b_sb.append(bt)

            deg_nm = cpool.tile([128, NT], f32, name="deg_nm")
            nc.sync.dma_start(deg_nm[:], deg_in.rearrange("(t p) -> p t", p=128))
            sq_nm = cpool.tile([128, NT], f32, name="sq_nm")
            nc.scalar.sqrt(sq_nm[:], deg_nm[:])
            dinv_nm = cpool.tile([128, NT], f32, name="dinv_nm")
            nc.vector.reciprocal(dinv_nm[:], sq_nm[:])
            dinv2_nm = cpool.tile([128, NT], f32, name="dinv2_nm")
            nc.vector.tensor_mul(dinv2_nm[:], dinv_nm[:], dinv_nm[:])

            deg_row = cpool.tile([1, PT], f32, name="deg_row")
            nc.sync.dma_start(deg_row[:], deg_in[None, :])
            sqd_f = cpool.tile([1, PT], f32, name="sqd_f")
            nc.scalar.sqrt(sqd_f[:], deg_row[:])
            sqd_row = cpool.tile([1, PT], bf16, name="sqd_row")
            nc.vector.tensor_copy(sqd_row[:], sqd_f[:])

            rg = [list(range(n_cores))]
            zpad = cpool.tile([PR - P, h], bf16, name="zpad")
            nc.vector.memset(zpad[:], 0.0)
            for L in (0, 1):
                nc.sync.dma_start(ag_in[L][P:PR, :], zpad[:])

            def mm_into(ps, Rb, t, wi, start=True):
                for c in range(KC):
                    nc.tensor.matmul(
                        ps[:],
                        lhsT=Rb[:, c, :],
                        rhs=w_sb[wi][:, c, :],
                        start=(start and c == 0),
                        stop=False,
                    )
                nc.tensor.matmul(
                    ps[:],
                    lhsT=sqd_row[0:1, t * 128 : (t + 1) * 128],
                    rhs=b_sb[wi][:],
                    start=False,
                    stop=True,
                )

            def emit_group(L, t, R):
                rows = min(128, P - t * 128)
                if rows <= 0:
                    return
                Rb = rbpool.tile([128, KC, 128], bf16, tag="Rb",
                                 name=f"Rb{L}_{t}")
                # DVE, not ACT: keeps the per-group emit pipeline
                # DVE(Rb) -> PE(mm) -> ACT(epilogue) one-stage-per-engine
                nc.vector.tensor_copy(Rb[:], R[:, :, t * 128 : (t + 1) * 128])
                if L < 2:
                    ps = ppool.tile([128, h], f32, tag="ps", name=f"ps{L}_{t}")
                    mm_into(ps, Rb, t, L)
                    tt = wpool.tile([128, h], bf16, tag="tt", name=f"tt{L}_{t}")
                    nc.scalar.activation(
                        tt[:], ps[:], AF.Relu, scale=dinv2_nm[:, t : t + 1]
                    )
                    nc.sync.dma_start(
                        ag_in[L][t * 128 : t * 128 + rows, :], tt[:rows, :]
                    )
                else:
                    ps2 = ppool.tile([128, h], f32, tag="ps", name=f"ps2_{t}")
                    mm_into(ps2, Rb, t, 2)
                    ps3 = ppool.tile([128, h], f32, tag="ps", name=f"ps3_{t}")
                    mm_into(ps3, Rb, t, 3)
                    o2 = wpool.tile([128, h], bf16, tag="tt", name=f"o2_{t}")
                    nc.scalar.activation(
                        o2[:], ps2[:], AF.Copy, scale=dinv_nm[:, t : t + 1]
                    )
                    nc.sync.dma_start(
                        out2_ext[t * 128 : t * 128 + rows, :], o2[:rows, :]
                    )
                    o3 = wpool.tile([128, h], bf16, tag="tt", name=f"o3_{t}")
                    nc.scalar.activation(
                        o3[:], ps3[:], AF.Copy, scale=dinv_nm[:, t : t + 1]
                    )
                    nc.sync.dma_start(
                        out3_ext[t * 128 : t * 128 + rows, :], o3[:rows, :]
                    )

            def process_layer(L):
                src = tbl0_in if L == 0 else tables[L]
                if L > 0 and collective:
                    nc.gpsimd.collective_compute(
                        "AllGather",
                        mybir.AluOpType.bypass,
                        replica_groups=rg,
                        ins=[ag_in[L - 1].opt()],
                        outs=[tables[L].opt()],
                    )
                R = rpool.tile([128, KC, PT], f32, tag="R", name=f"R{L}")
                remaining = list(seg_cnt)
                ioff = 0
                for ci, (n_idx, segs) in enumerate(calls):
                    gt = gpool.tile([128, KC, n_idx], bf16, tag="gt",
                                    name=f"gt{L}_{ci}")
                    nc.gpsimd.dma_gather(
                        gt[:],
                        src[:, :],
                        gidx[:, ioff // 16 : (ioff + n_idx) // 16],
                        n_idx,
                        n_idx,
                        h,
                        transpose=True,
                        single_packet=(n_idx <= 896),
                    )
                    ioff += n_idx
                    for soff, nw, K, dest_start in segs:
                        nd = nw * WIN
                        g4 = gt[:, :, soff : soff + nd * K].rearrange(
                            "p c (d k) -> p c d k", k=K
                        )
                        k = K
                        while k > 2:
                            m = k // 2
                            nc.vector.tensor_add(
                                g4[:, :, :, 0:m],
                                g4[:, :, :, 0:m],
                                g4[:, :, :, k - m : k],
                            )
                            k -= m
                        nc.vector.tensor_add(
                            R[:, :, dest_start : dest_start + nd],
                            g4[:, :, :, 0:1].rearrange("p c d k -> p c (d k)"),
                            g4[:, :, :, 1:2].rearrange("p c d k -> p c (d k)"),
                        )
                        g0 = dest_start // 128
                        g1 = (dest_start + nd - 1) // 128
                        for g in range(g0, g1 + 1):
                            remaining[g] -= 1
                            if remaining[g] == 0:
                                emit_group(L, g, R)

            for L in range(3):
                process_layer(L)

    nc.compile()
    return nc


# -------------------- public entry --------------------
def kernel(x, edge_index, W1, b1, W1_1, b1_1, W2, b2, W3, b3):
    from concourse.bass_utils import run_bass_kernel_spmd

    x = np.asarray(x, dtype=np.float32)
    edge_index = np.asarray(edge_index, dtype=np.int32)
    n_nodes, h = x.shape
    meta = _prep_graph(edge_index, n_nodes, C)
    P, PT, TOT, PR, NTAB = (meta["P"], meta["PT"], meta["TOT"], meta["PR"],
                            meta["NTAB"])

    key = (n_nodes, h, TOT, tuple(meta["Kw"]))
    if key not in _KERNEL_CACHE:
        _KERNEL_CACHE[key] = _build_bass(
            n_nodes, C, h, P, PT, TOT, PR, NTAB, meta["calls"],
        )
    nc = _KERNEL_CACHE[key]

    # host-built layer-1 table: dinv * x, permuted to new ids, bf16, padded
    oon = meta["orig_of_new"]
    deg_full = np.bincount(
        np.concatenate([edge_index[1].astype(np.int64),
                        np.arange(n_nodes, dtype=np.int64)]),
        minlength=n_nodes,
    ).astype(np.float64)
    dinv = 1.0 / np.sqrt(deg_full)
    t0 = (x.astype(np.float64) * dinv[:, None])[oon]
    table0 = np.zeros((NTAB, h), dtype=np.float32)
    for c in range(C):
        table0[c * PR : c * PR + P] = t0[c * P : (c + 1) * P]
    table0 = _to_bf16(table0)

    Ws = {"W1": W1, "W1_1": W1_1, "W2": W2, "W3": W3}
    bs = {"b1": b1, "b1_1": b1_1, "b2": b2, "b3": b3}
    in_maps = []
    for c in range(C):
        m = {
            "table0": table0,
            "deg_loc": meta["deg_loc"][c],
            "gidx": np.ascontiguousarray(meta["gidx"][c]),
        }
        for k, v in Ws.items():
            m[k] = _to_bf16(np.ascontiguousarray(v, dtype=np.float32))
        for k, v in bs.items():
            m[k] = _to_bf16(np.ascontiguousarray(v, dtype=np.float32))
        in_maps.append(m)

    global LAST_RESULTS
    LAST_RESULTS = run_bass_kernel_spmd(nc, in_maps, core_ids=list(range(C)))
    res = LAST_RESULTS.results

    out2_new = np.concatenate(
        [_from_bf16(res[c]["out2"]) for c in range(C)], axis=0)
    out3_new = np.concatenate(
        [_from_bf16(res[c]["out3"]) for c in range(C)], axis=0)
    new_id = meta["new_id"]
    return out2_new[new_id].astype(np.float32), out3_new[new_id].astype(np.float32)


def _to_bf16(a):
    import ml_dtypes
    return a.astype(ml_dtypes.bfloat16)


def _from_bf16(a):
    return np.asarray(a, dtype=np.float32)


# revision 38
# speedup vs baseline: 1.2932x; 1.0373x over previous
"""GCNEncoder Trainium2 kernel (8 NeuronCores, SPMD).

Strategy (graph/data parallel, per sharding hint):
  - Nodes dealt round-robin-by-degree across 8 cores (2500 each); [H,H]
    weights replicated (shipped bf16); deg-derived scale vectors
    (dinv, dinv^2, sqrt(deg)) precomputed host-side as tiny inputs.
  - The layer-1 table (dinv * x, bf16) is computed host-side and passed
    as an input, so only 2 AllGathers remain (out2/out3 share the
    layer-3 aggregation).
  - Per layer: gather in-edge source rows from the bf16 table with
    transposed dma_gather (feature-major [128, 2, slots]); slots laid out
    in 16-destination windows with exact per-window K = max in-degree
    (ascending-K stream, minimal padding); each gather call (<=2176 idxs)
    spans many windows.
  - Segment sum on DVE: ceil-halving fold chains (2x-mode adds; top half
    folded onto bottom half, odd middle slot passes through), final pair
    written into the layer R tile (bf16, feature-major) which the PE
    reads directly as lhsT.  One DVE op per fold level per same-K run
    inside a gather call, so op count stays small.  Groups of 128 dests
    emit (matmul + fused epilogue) as soon as their segments land.
  - agg(x) @ W == agg(x @ W): one aggregation per layer feeds the [HxH]
    matmul; the two output heads share one matmul pass via a fused
    [W2|W3] rhs into a 512-wide PSUM.  norm = dinv[row]*dinv[col] folds
    into the table pre-scale and a per-dest post-scale fused into the
    PSUM->SBUF activation; bias enters as a rank-1 (sqrt(deg) x b)
    matmul.

Self-contained: hardcodes problem shapes; needs only numpy + concourse.
"""

import numpy as np

# -------------------- problem constants --------------------
N_NODES = 20000
N_EDGES = 320000
H = 256
C = 8  # cores
WIN = 16  # dests per K-window
MAXI = 2176  # max idxs per dma_gather call (descriptor ring)
SCRATCH = 49152

_KERNEL_CACHE = {}
LAST_RESULTS = None


# -------------------- host-side graph prep --------------------
def _prep_graph(edge_index, n_nodes, n_cores):
    P = n_nodes // n_cores
    row = edge_index[0].astype(np.int64)
    col = edge_index[1].astype(np.int64)
    loop = np.arange(n_nodes, dtype=np.int64)
    row_f = np.concatenate([row, loop])
    col_f = np.concatenate([col, loop])
    deg = np.bincount(col_f, minlength=n_nodes).astype(np.int64)  # >= 1

    # deal nodes round-robin by ascending degree
    order = np.argsort(deg, kind="stable")
    pos = np.empty(n_nodes, dtype=np.int64)
    pos[order] = np.arange(n_nodes)
    new_id = (pos % n_cores) * P + pos // n_cores  # old -> new
    orig_of_new = np.empty(n_nodes, dtype=np.int64)
    orig_of_new[new_id] = np.arange(n_nodes)

    src_new = new_id[row_f]
    dst_new = new_id[col_f]

    PT = ((P + 127) // 128) * 128
    NW = PT // WIN
    PR = P + 16  # table rows per rank (16 zero pad rows)
    ZROW = P  # rank0's first pad row == all-zero table row
    NTAB = n_cores * PR

    deg_new = deg[orig_of_new]
    deg_loc = np.ones((n_cores, PT), dtype=np.float32)
    for c in range(n_cores):
        deg_loc[c, :P] = deg_new[c * P : (c + 1) * P]

    # per-window K: max degree over all cores in the window (exact)
    dl = np.ones((n_cores, PT), dtype=np.int64)
    for c in range(n_cores):
        dl[c, :P] = deg_new[c * P : (c + 1) * P]
    Kw = np.zeros(NW, dtype=np.int64)
    for w in range(NW):
        Kw[w] = max(int(dl[:, w * WIN : (w + 1) * WIN].max()), 2)

    # stream: natural window order (ascending K).  The last gather call then
    # holds the few highest-K windows (~1 dest group), so the per-layer
    # drain (final fold -> matmul -> write chain) is short.  All-pad windows
    # (no real dest) are not gathered at all.
    stream = [w for w in range(NW) if w * WIN < P]

    # pack gather calls: whole windows, n_idx <= MAXI, pad to %128 with ZROW
    calls_raw = []
    cur, cur_n = [], 0
    for w in stream:
        wn = WIN * int(Kw[w])
        if cur_n + wn > MAXI and cur:
            calls_raw.append((cur_n + (-cur_n) % 128, cur))
            cur, cur_n = [], 0
        cur.append((w, cur_n))
        cur_n += wn
    if cur:
        calls_raw.append((cur_n + (-cur_n) % 128, cur))

    # slot stream offsets per window
    TOT = 0
    woff = {}
    for n_idx, ws in calls_raw:
        for w, off in ws:
            woff[w] = TOT + off
        TOT += n_idx
    dest_base = np.zeros(PT, dtype=np.int64)
    for w in stream:
        d = np.arange(WIN)
        dest_base[w * WIN : (w + 1) * WIN] = woff[w] + d * Kw[w]

    # segments: same-K ascending-window runs within one call
    calls = []
    for n_idx, ws in calls_raw:
        segs = []
        i = 0
        while i < len(ws):
            w0, off0 = ws[i]
            j = i
            while (
                j + 1 < len(ws)
                and ws[j + 1][0] == ws[j][0] + 1
                and Kw[ws[j + 1][0]] == Kw[w0]
            ):
                j += 1
            segs.append((off0, j - i + 1, int(Kw[w0]), w0 * WIN))
            i = j + 1
        calls.append((int(n_idx), segs))

    # fill slots
    slots = np.full((n_cores, TOT), ZROW, dtype=np.int64)
    src_trow = (src_new // P) * PR + (src_new % P)
    e_core = dst_new // P
    e_dloc = dst_new % P
    sort_k = np.argsort(e_core * n_nodes + e_dloc, kind="stable")
    sc, sd, ss = e_core[sort_k], e_dloc[sort_k], src_trow[sort_k]
    key = sc * n_nodes + sd
    first = np.r_[True, key[1:] != key[:-1]]
    run_start = np.maximum.accumulate(np.where(first, np.arange(key.size), 0))
    rank = np.arange(key.size) - run_start
    flat = dest_base[sd] + rank
    slots[sc, flat] = ss

    # wrap to [128, TOT//16] int16
    assert TOT % 16 == 0
    wrapped = np.empty((n_cores, 128, TOT // 16), dtype=np.int16)
    for c in range(n_cores):
        w16 = slots[c].reshape(TOT // 16, 16).T.astype(np.int16)
        wrapped[c] = np.tile(w16, (8, 1))

    return dict(
        P=P, PT=PT, NW=NW, TOT=TOT, ZROW=ZROW, PR=PR, NTAB=NTAB,
        Kw=[int(k) for k in Kw], calls=calls,
        new_id=new_id, orig_of_new=orig_of_new,
        deg_loc=deg_loc, gidx=wrapped,
    )


# -------------------- bass kernel builder --------------------
def _build_bass(n_nodes, n_cores, h, P, PT, TOT, PR, NTAB, calls,
                collective=True):
    import concourse.bacc as bacc
    import concourse.mybir as mybir
    import concourse.tile as tile

    dt = mybir.dt
    f32, bf16, i16 = dt.float32, dt.bfloat16, dt.int16
    AF = mybir.ActivationFunctionType
    NT = PT // 128  # 128-dest groups per core
    KC = h // 128  # 2

    nc = bacc.Bacc(dynamic_dma_scratch_size=SCRATCH)
    tbl0_in = nc.declare_dram_parameter("table0", [NTAB, h], bf16, isOutput=False)
    dinv_in = nc.declare_dram_parameter("dinv_nm", [128, PT // 128], f32,
                                        isOutput=False)
    dinv2_in = nc.declare_dram_parameter("dinv2_nm", [128, PT // 128], f32,
                                         isOutput=False)
    sqd_in = nc.declare_dram_parameter("sqd_row", [1, PT], bf16, isOutput=False)
    idx_in = nc.declare_dram_parameter("gidx", [128, TOT // 16], i16, isOutput=False)
    W_in = [nc.declare_dram_parameter(nm, [h, h], bf16, isOutput=False)
            for nm in ("W1", "W1_1", "W2", "W3")]
    b_in = [nc.declare_dram_parameter(nm, [h], bf16, isOutput=False)
            for nm in ("b1", "b1_1", "b2", "b3")]
    out2_ext = nc.declare_dram_parameter("out2", [P, h], bf16, isOutput=True)
    out3_ext = nc.declare_dram_parameter("out3", [P, h], bf16, isOutput=True)

    # per-group segment counts (for emit scheduling)
    seg_cnt = [0] * NT
    for n_idx, segs in calls:
        for soff, nw, K, dest_start in segs:
            g0 = dest_start // 128
            g1 = (dest_start + nw * WIN - 1) // 128
            for g in range(g0, g1 + 1):
                seg_cnt[g] += 1

    with tile.TileContext(nc) as tc:
        with (
            tc.tile_pool(name="dram", bufs=1, space="DRAM") as dpool,
            tc.tile_pool(name="const", bufs=1) as cpool,
            tc.tile_pool(name="gather", bufs=5) as gpool,
            tc.tile_pool(name="rlay", bufs=2) as rpool,
            tc.tile_pool(name="work", bufs=4) as wpool,
            tc.tile_pool(name="psum", bufs=6, space="PSUM") as ppool,
            tc.tile_pool(name="psumw", bufs=2, space="PSUM") as pwpool,
        ):
            # gidx first slice + small deg-derived consts lead the DMA queue
            gidx = cpool.tile([128, TOT // 16], i16, name="gidx_sb")
            n0 = calls[0][0]
            nc.sync.dma_start(gidx[:, : n0 // 16], idx_in[:, : n0 // 16])
            dinv_nm = cpool.tile([128, NT], f32, name="dinv_nm")
            nc.sync.dma_start(dinv_nm[:], dinv_in[:])
            dinv2_nm = cpool.tile([128, NT], f32, name="dinv2_nm")
            nc.sync.dma_start(dinv2_nm[:], dinv2_in[:])
            sqd_row = cpool.tile([1, PT], bf16, name="sqd_row")
            nc.sync.dma_start(sqd_row[:], sqd_in[:])
            nc.sync.dma_start(gidx[:, n0 // 16 :], idx_in[:, n0 // 16 :])

            # internal DRAM: AG inputs (layer 1,2 tables)
            ag_in = [dpool.tile([PR, h], bf16, name=f"agin{L}") for L in (1, 2)]
            if collective:
                tables = [None,
                          dpool.tile([NTAB, h], bf16, addr_space="Shared",
                                     name="table1"),
                          dpool.tile([NTAB, h], bf16, addr_space="Shared",
                                     name="table2")]
            else:
                tables = [None,
                          nc.declare_dram_parameter("tbl1", [NTAB, h], bf16,
                                                    isOutput=False),
                          nc.declare_dram_parameter("tbl2", [NTAB, h], bf16,
                                                    isOutput=False)]

            # ---- constants ----
            w_sb = []
            for i in range(2):
                wb = cpool.tile([128, KC, h], bf16, name=f"wb{i}")
                nc.sync.dma_start(wb[:], W_in[i].rearrange("(c p) j -> p c j", p=128))
                w_sb.append(wb)
            b_sb = []
            for i in range(2):
                bt = cpool.tile([1, h], bf16, name=f"bv{i}")
                nc.sync.dma_start(bt[:], b_in[i][None, :])
                b_sb.append(bt)
            # fused [W2|W3] / [b2|b3] for the two output heads (shared lhsT)
            w23 = cpool.tile([128, KC, 2 * h], bf16, name="w23")
            nc.sync.dma_start(w23[:, :, 0:h],
                              W_in[2].rearrange("(c p) j -> p c j", p=128))
            nc.sync.dma_start(w23[:, :, h : 2 * h],
                              W_in[3].rearrange("(c p) j -> p c j", p=128))
            b23 = cpool.tile([1, 2 * h], bf16, name="b23")
            nc.sync.dma_start(b23[:, 0:h], b_in[2][None, :])
            nc.sync.dma_start(b23[:, h : 2 * h], b_in[3][None, :])

            rg = [list(range(n_cores))]
            zpad = cpool.tile([PR - P, h], bf16, name="zpad")
            nc.vector.memset(zpad[:], 0.0)
            for L in (0, 1):
                nc.sync.dma_start(ag_in[L][P:PR, :], zpad[:])

            def mm_into(ps, Rb, t, wi, start=True):
                for c in range(KC):
                    nc.tensor.matmul(
                        ps[:],
                        lhsT=Rb[:, c, :],
                        rhs=w_sb[wi][:, c, :],
                        start=(start and c == 0),
                        stop=False,
                    )
                nc.tensor.matmul(
                    ps[:],
                    lhsT=sqd_row[0:1, t * 128 : (t + 1) * 128],
                    rhs=b_sb[wi][:],
                    start=False,
                    stop=True,
                )

            def emit_group(L, t, R):
                rows = min(128, P - t * 128)
                if rows <= 0:
                    return
                Rb = R[:, :, t * 128 : (t + 1) * 128]
                if L < 2:
                    ps = ppool.tile([128, h], f32, tag="ps", name=f"ps{L}_{t}")
                    mm_into(ps, Rb, t, L)
                    tt = wpool.tile([128, h], bf16, tag="tt", name=f"tt{L}_{t}")
                    nc.scalar.activation(
                        tt[:], ps[:], AF.Relu, scale=dinv2_nm[:, t : t + 1]
                    )
                    nc.sync.dma_start(
                        ag_in[L][t * 128 : t * 128 + rows, :], tt[:rows, :]
                    )
                else:
                    ps = pwpool.tile([128, 2 * h], f32, tag="psw",
                                    name=f"psw_{t}")
                    for c in range(KC):
                        nc.tensor.matmul(
                            ps[:], lhsT=Rb[:, c, :], rhs=w23[:, c, :],
                            start=(c == 0), stop=False,
                        )
                    nc.tensor.matmul(
                        ps[:],
                        lhsT=sqd_row[0:1, t * 128 : (t + 1) * 128],
                        rhs=b23[:],
                        start=False, stop=True,
                    )
                    o2 = wpool.tile([128, h], bf16, tag="tt", name=f"o2_{t}")
                    nc.scalar.activation(
                        o2[:], ps[:, 0:h], AF.Copy, scale=dinv_nm[:, t : t + 1]
                    )
                    nc.sync.dma_start(
                        out2_ext[t * 128 : t * 128 + rows, :], o2[:rows, :]
                    )
                    o3 = wpool.tile([128, h], bf16, tag="tt", name=f"o3_{t}")
                    nc.scalar.activation(
                        o3[:], ps[:, h : 2 * h], AF.Copy,
                        scale=dinv_nm[:, t : t + 1]
                    )
                    nc.sync.dma_start(
                        out3_ext[t * 128 : t * 128 + rows, :], o3[:rows, :]
                    )

            def process_layer(L):
                src = tbl0_in if L == 0 else tables[L]
                if L > 0 and collective:
                    nc.gpsimd.collective_compute(
                        "AllGather",
                        mybir.AluOpType.bypass,
                        replica_groups=rg,
                        ins=[ag_in[L - 1].opt()],
                        outs=[tables[L].opt()],
                    )
                R = rpool.tile([128, KC, PT], bf16, tag="R", name=f"R{L}")
                remaining = list(seg_cnt)
                ioff = 0
                for ci, (n_idx, segs) in enumerate(calls):
                    gt = gpool.tile([128, KC, n_idx], bf16, tag="gt",
                                    name=f"gt{L}_{ci}")
                    nc.gpsimd.dma_gather(
                        gt[:],
                        src[:, :],
                        gidx[:, ioff // 16 : (ioff + n_idx) // 16],
                        n_idx,
                        n_idx,
                        h,
                        transpose=True,
                        single_packet=(n_idx <= 896),
                    )
                    ioff += n_idx
                    # fold levels emitted round-robin across the call's
                    # segments: consecutive DVE ops are independent, so the
                    # engine never stalls on its own in-place chain
                    state = []
                    for soff, nw, K, dest_start in segs:
                        nd = nw * WIN
                        g4 = gt[:, :, soff : soff + nd * K].rearrange(
                            "p c (d k) -> p c d k", k=K
                        )
                        state.append([g4, K, nd, dest_start])
                    live = list(range(len(state)))
                    while live:
                        nxt = []
                        for si in live:
                            g4, k, nd, dest_start = state[si]
                            if k > 2:
                                m = k // 2
                                nc.vector.tensor_add(
                                    g4[:, :, :, 0:m],
                                    g4[:, :, :, 0:m],
                                    g4[:, :, :, k - m : k],
                                )
                                state[si][1] = k - m
                                nxt.append(si)
                                continue
                            nc.vector.tensor_add(
                                R[:, :, dest_start : dest_start + nd],
                                g4[:, :, :, 0:1].rearrange(
                                    "p c d k -> p c (d k)"),
                                g4[:, :, :, 1:2].rearrange(
                                    "p c d k -> p c (d k)"),
                            )
                            g0 = dest_start // 128
                            g1 = (dest_start + nd - 1) // 128
                            for g in range(g0, g1 + 1):
                                remaining[g] -= 1
                                if remaining[g] == 0:
                                    emit_group(L, g, R)
                        live = nxt

            for L in range(3):
                process_layer(L)

    nc.compile()
    return nc


# -------------------- public entry --------------------
def kernel(x, edge_index, W1, b1, W1_1, b1_1, W2, b2, W3, b3):
    from concourse.bass_utils import run_bass_kernel_spmd

    x = np.asarray(x, dtype=np.float32)
    edge_index = np.asarray(edge_index, dtype=np.int32)
    n_nodes, h = x.shape
    meta = _prep_graph(edge_index, n_nodes, C)
    P, PT, TOT, PR, NTAB = (meta["P"], meta["PT"], meta["TOT"], meta["PR"],
                            meta["NTAB"])

    key = (n_nodes, h, TOT, tuple(meta["Kw"]))
    if key not in _KERNEL_CACHE:
        _KERNEL_CACHE[key] = _build_bass(
            n_nodes, C, h, P, PT, TOT, PR, NTAB, meta["calls"],
        )
    nc = _KERNEL_CACHE[key]

    # host-built layer-1 table: dinv * x, permuted to new ids, bf16, padded
    oon = meta["orig_of_new"]
    deg_full = np.bincount(
        np.concatenate([edge_index[1].astype(np.int64),
                        np.arange(n_nodes, dtype=np.int64)]),
        minlength=n_nodes,
    ).astype(np.float64)
    dinv = 1.0 / np.sqrt(deg_full)
    t0 = (x.astype(np.float64) * dinv[:, None])[oon]
    table0 = np.zeros((NTAB, h), dtype=np.float32)
    for c in range(C):
        table0[c * PR : c * PR + P] = t0[c * P : (c + 1) * P]
    table0 = _to_bf16(table0)

    Ws = {"W1": W1, "W1_1": W1_1, "W2": W2, "W3": W3}
    bs = {"b1": b1, "b1_1": b1_1, "b2": b2, "b3": b3}
    NT = PT // 128
    in_maps = []
    for c in range(C):
        dl = meta["deg_loc"][c].astype(np.float64)
        dinv_c = (1.0 / np.sqrt(dl)).astype(np.float32)
        m = {
            "table0": table0,
            "dinv_nm": np.ascontiguousarray(dinv_c.reshape(NT, 128).T),
            "dinv2_nm": np.ascontiguousarray(
                (dinv_c * dinv_c).reshape(NT, 128).T),
            "sqd_row": _to_bf16(np.sqrt(dl)[None, :].astype(np.float32)),
            "gidx": np.ascontiguousarray(meta["gidx"][c]),
        }
        for k, v in Ws.items():
            m[k] = _to_bf16(np.ascontiguousarray(v, dtype=np.float32))
        for k, v in bs.items():
            m[k] = _to_bf16(np.ascontiguousarray(v, dtype=np.float32))
        in_maps.append(m)

    global LAST_RESULTS
    LAST_RESULTS = run_bass_kernel_spmd(nc, in_maps, core_ids=list(range(C)))
    res = LAST_RESULTS.results

    out2_new = np.concatenate(
        [_from_bf16(res[c]["out2"]) for c in range(C)], axis=0)
    out3_new = np.concatenate(
        [_from_bf16(res[c]["out3"]) for c in range(C)], axis=0)
    new_id = meta["new_id"]
    return out2_new[new_id].astype(np.float32), out3_new[new_id].astype(np.float32)


def _to_bf16(a):
    import ml_dtypes
    return a.astype(ml_dtypes.bfloat16)


def _from_bf16(a):
    return np.asarray(a, dtype=np.float32)
